# revision 12
# baseline (speedup 1.0000x reference)
"""Transformer decoder (nn_Decoder) Bass kernel for 8 TRN2 NeuronCores.

Strategy: data-parallel over batch (B=8 -> one batch element per core, no
collectives). Per core the full 6-layer decoder runs on-chip:

- activations live in SBUF feature-major (x^T: [D partitions, tokens free])
- attention uses the transposed-scores layout S^T[k, q] so softmax
  denominators come free from a ones-column appended to V (row 64 of the
  AV matmul output), and exp() is a single ScalarE pass PSUM->SBUF
- heads are packed in pairs: contraction dim dk=64 -> two heads occupy
  row/col halves of the PE array via tile_position
- layernorm stats (feature-dim = partition-dim reductions) via ones-vector
  matmuls on the PE; per-token broadcast via gpsimd partition_broadcast
- embedding lookup via indirect DMA gather; causal mask applied as a
  compile-time block-skip + a 4-variant diagonal tril multiply (the mask
  tensor is inspected on the host; a generic mask path covers non-causal)

dtypes: float32r (full-rate fp32 on the PE, ~11-bit mantissa) for
projections/FFN1/LN/residual; bf16 for the attention core (Q^T,K^T,P,V)
and FFN2 (w2, h) to fit SBUF.
"""

import numpy as np

import concourse.bass as bass
import concourse.bacc as bacc
import concourse.mybir as mybir
import concourse.tile as tile
from concourse.bass_utils import run_bass_kernel_spmd
from concourse.masks import make_identity

P = 128
B = 8
T = 1023
T1 = 1024  # decoder length (T+1)
D = 512
H = 8
DK = 64
L = 6
DFF = 2048
S = 1024  # encoder length
V = 10000
NPOS = 2048
EPS = 1e-6

F32 = mybir.dt.float32
F32R = mybir.dt.float32r
BF16 = mybir.dt.bfloat16
I32 = mybir.dt.int32

AF = mybir.ActivationFunctionType
ALU = mybir.AluOpType

NCH = D // P  # 4 feature chunks
NTT = T1 // P  # 8 token tiles
NQC = T1 // 512  # 2 query chunks
NPAIR = H // 2  # 4 head pairs
NDB = DFF // P  # 16 dff blocks


def _sinusoid_table(n_position, d_hid):
    pos = np.arange(n_position)[:, None].astype(np.float64)
    i = np.arange(d_hid)[None, :]
    angle = pos / np.power(10000.0, 2 * (i // 2) / d_hid)
    tab = np.zeros((n_position, d_hid))
    tab[:, 0::2] = np.sin(angle[:, 0::2])
    tab[:, 1::2] = np.cos(angle[:, 1::2])
    return tab.astype(np.float32)


def _build_program(causal, src_ones, ln_id, b1_zero, b2_zero, n_sub=None):
    # n_sub: debug — stop after this many sublayers (3 per layer)
    nc = bacc.Bacc()

    # ---------------- DRAM parameters (per core) ----------------
    idx_in = nc.declare_dram_parameter("idx", [P, NTT], I32, isOutput=False)
    emb_in = nc.declare_dram_parameter("emb", [V, D], F32, isOutput=False)
    pos_in = nc.declare_dram_parameter("pos", [P, NTT, D], F32, isOutput=False)
    enc_in = nc.declare_dram_parameter("encT", [P, NCH, S], BF16, isOutput=False)
    tril_in = nc.declare_dram_parameter("tril4", [P, 4, 512], BF16, isOutput=False)

    wq_s = nc.declare_dram_parameter("wq_s", [L, D, D], F32R, isOutput=False)
    wk_s = nc.declare_dram_parameter("wk_s", [L, D, D], F32R, isOutput=False)
    wv_s = nc.declare_dram_parameter("wv_s", [L, D, D], F32R, isOutput=False)
    wo_s = nc.declare_dram_parameter("wo_s", [L, D, D], F32R, isOutput=False)
    wq_c = nc.declare_dram_parameter("wq_c", [L, D, D], F32R, isOutput=False)
    wk_c = nc.declare_dram_parameter("wk_c", [L, D, D], BF16, isOutput=False)
    wv_c = nc.declare_dram_parameter("wv_c", [L, D, D], BF16, isOutput=False)
    wo_c = nc.declare_dram_parameter("wo_c", [L, D, D], F32R, isOutput=False)
    w1_in = nc.declare_dram_parameter("w1", [L, D, DFF], F32R, isOutput=False)
    w2_in = nc.declare_dram_parameter("w2", [L, DFF, D], BF16, isOutput=False)
    b1_in = nc.declare_dram_parameter("b1", [L, P, NDB], F32, isOutput=False)
    b2_in = nc.declare_dram_parameter("b2", [L, P, NCH], F32, isOutput=False)

    if not ln_id:
        ln0g_in = nc.declare_dram_parameter("ln0_g", [1, D], F32, isOutput=False)
        ln0b_in = nc.declare_dram_parameter("ln0_b", [1, D], F32, isOutput=False)
        lng_in = nc.declare_dram_parameter("lng", [L, 3, P, NCH], F32, isOutput=False)
        lnb_in = nc.declare_dram_parameter("lnb", [L, 3, P, NCH], F32, isOutput=False)
    if not causal:
        mask_in = nc.declare_dram_parameter("mask", [T1, T1], I32, isOutput=False)
    if not src_ones:
        smask_in = nc.declare_dram_parameter("smask", [P, NTT], I32, isOutput=False)

    out_dram = nc.declare_dram_parameter("out", [P, NTT, D], F32, isOutput=True)

    from contextlib import ExitStack

    with tile.TileContext(nc) as tc, ExitStack() as _es:
        def _pool(**kw):
            return _es.enter_context(tc.tile_pool(**kw))

        if True:
            constp = _pool(name="const", bufs=1)
            xresp = _pool(name="xres", bufs=2)
            qkp = _pool(name="qk", bufs=1)
            vaugp = _pool(name="vaug", bufs=1)
            onormp = _pool(name="onorm", bufs=1)
            ptp = _pool(name="pt", bufs=3)
            watp = _pool(name="wat", bufs=3)
            wcbp = _pool(name="wcb", bufs=2)
            w1p = _pool(name="w1p", bufs=2)
            w2p = _pool(name="w2p", bufs=1)
            hp = _pool(name="hp", bufs=16)
            lnsp = _pool(name="lns", bufs=2)
            lnbp = _pool(name="lnb", bufs=1)
            statp = _pool(name="stat", bufs=1)
            recp = _pool(name="rec", bufs=2)
            biasp = _pool(name="bias", bufs=2)
            pp = _pool(name="pp", bufs=2, space="PSUM")
            psSp = _pool(name="psS", bufs=2, space="PSUM")
            psOp = _pool(name="psO", bufs=2, space="PSUM")
            ident = constp.tile([P, P], F32, tag="ident")
            make_identity(nc, ident[:])
            onesf32 = constp.tile([P, 1], F32, tag="onesf32")
            nc.vector.memset(onesf32[:], 1.0)
            onesf = constp.tile([P, 1], F32R, tag="onesf")
            nc.vector.tensor_copy(onesf[:], onesf32[:])
            epst = constp.tile([P, 1], F32, tag="epst")
            nc.vector.memset(epst[:], EPS)
            tril4 = constp.tile([P, 4, 512], BF16, tag="tril4")
            nc.sync.dma_start(tril4[:], tril_in[:])
            if not src_ones:
                smask_i = constp.tile([P, NTT], I32, tag="smi")
                nc.sync.dma_start(smask_i[:], smask_in[:])
                smask_f = constp.tile([P, NTT], F32, tag="smf")
                nc.vector.tensor_copy(smask_f[:], smask_i[:])

            encT = constp.tile([P, NCH, S], BF16, tag="encT")
            nc.sync.dma_start(encT[:], enc_in[:])

            # ================= stage 0: embed + LN0 + transpose =================
            xT = xresp.tile([P, NCH, T1], F32R, tag="x")
            with ExitStack() as _es0:
                st0 = _es0.enter_context(tc.tile_pool(name="st0", bufs=2))
                st0s = _es0.enter_context(tc.tile_pool(name="st0s", bufs=2))
                idxt = st0s.tile([P, NTT], I32, tag="idx")
                nc.sync.dma_start(idxt[:], idx_in[:])
                if not ln_id:
                    g0row = st0s.tile([1, D], F32, tag="g0r")
                    nc.sync.dma_start(g0row[:], ln0g_in[:])
                    b0row = st0s.tile([1, D], F32, tag="b0r")
                    nc.sync.dma_start(b0row[:], ln0b_in[:])
                    g0b = st0s.tile([P, D], F32, tag="g0b")
                    nc.gpsimd.partition_broadcast(g0b[:], g0row[:])
                    b0b = st0s.tile([P, D], F32, tag="b0b")
                    nc.gpsimd.partition_broadcast(b0b[:], b0row[:])
                for t in range(NTT):
                    gat = st0.tile([P, D], F32, tag="gat")
                    nc.gpsimd.indirect_dma_start(
                        out=gat[:],
                        out_offset=None,
                        in_=emb_in[:],
                        in_offset=bass.IndirectOffsetOnAxis(ap=idxt[:, t : t + 1], axis=0),
                    )
                    if t == 0:
                        nc.vector.memset(gat[0:1, :], 0.0)
                    post = st0.tile([P, D], F32, tag="pos")
                    nc.sync.dma_start(post[:], pos_in[:, t, :])
                    x0 = gat
                    nc.vector.tensor_tensor(x0[:], gat[:], post[:], ALU.add)
                    # LN0 (token-major; per-token stats along free dim)
                    ssum = st0s.tile([P, 1], F32, tag="ssum")
                    nc.vector.tensor_reduce(ssum[:], x0[:], mybir.AxisListType.X, ALU.add)
                    xsq = st0.tile([P, D], F32, tag="pos")
                    nc.vector.tensor_tensor(xsq[:], x0[:], x0[:], ALU.mult)
                    ssq = st0s.tile([P, 1], F32, tag="ssq")
                    nc.vector.tensor_reduce(ssq[:], xsq[:], mybir.AxisListType.X, ALU.add)
                    mu = st0s.tile([P, 1], F32, tag="mu")
                    nc.vector.tensor_scalar_mul(mu[:], ssum[:], 1.0 / D)
                    e2 = st0s.tile([P, 1], F32, tag="e2")
                    nc.vector.tensor_scalar_mul(e2[:], ssq[:], 1.0 / D)
                    mu2 = st0s.tile([P, 1], F32, tag="mu2")
                    nc.vector.tensor_tensor(mu2[:], mu[:], mu[:], ALU.mult)
                    var = st0s.tile([P, 1], F32, tag="var")
                    nc.vector.tensor_tensor(var[:], e2[:], mu2[:], ALU.subtract)
                    sd = st0s.tile([P, 1], F32, tag="sd")
                    nc.scalar.activation(sd[:], var[:], AF.Sqrt, bias=epst[:])
                    rstd = st0s.tile([P, 1], F32, tag="rstd")
                    nc.vector.reciprocal(rstd[:], sd[:])
                    xn = st0.tile([P, D], F32, tag="xn")
                    nc.vector.tensor_scalar(
                        xn[:], x0[:], mu[:], rstd[:], ALU.subtract, ALU.mult
                    )
                    if not ln_id:
                        nc.vector.tensor_tensor(xn[:], xn[:], g0b[:], ALU.mult)
                        nc.vector.tensor_tensor(xn[:], xn[:], b0b[:], ALU.add)
                    for c in range(NCH):
                        pst = pp.tile([P, 512], F32, tag="pp")
                        nc.tensor.transpose(
                            pst[:, 0:P], xn[:, c * P : (c + 1) * P], ident[:]
                        )
                        nc.vector.tensor_copy(
                            xT[:, c, t * P : (t + 1) * P], pst[:, 0:P]
                        )

            # ================= helpers =================
            def load_w(dram, l, pool, dtype, tag):
                w = pool.tile([P, NCH, D], dtype, tag=tag)
                nc.sync.dma_start(
                    w[:], dram[l].rearrange("(c p) n -> p c n", p=P)
                )
                return w

            def feature_ln(R, lw=None):
                """R: [P, NCH, T1] f32r residual-sum -> new xT tile (f32r)."""
                xn = xresp.tile([P, NCH, T1], F32R, tag="x", name="xn")
                for qc in range(NQC):
                    sl = slice(qc * 512, qc * 512 + 512)
                    psm = pp.tile([P, 512], F32, tag="pp", name="psm")
                    psq = pp.tile([P, 512], F32, tag="pp", name="psq")
                    for c in range(NCH):
                        nc.tensor.matmul(
                            out=psm[0:1, :], lhsT=onesf[:], rhs=R[:, c, sl],
                            start=(c == 0), stop=(c == NCH - 1),
                        )
                    for c in range(NCH):
                        xsq = lnsp.tile([P, 512], F32R, tag="xsq", name="xsq")
                        nc.vector.tensor_tensor(
                            xsq[:], R[:, c, sl].bitcast(F32), R[:, c, sl].bitcast(F32),
                            ALU.mult,
                        )
                        nc.tensor.matmul(
                            out=psq[0:1, :], lhsT=onesf[:], rhs=xsq[:],
                            start=(c == 0), stop=(c == NCH - 1),
                        )
                    sums = statp.tile([1, 512], F32, tag="sums", name="sums")
                    nc.vector.tensor_copy(sums[:], psm[0:1, :])
                    # tmp = ssq - sums^2/D ; rsd = 1/sqrt(tmp/D + eps)
                    tmp = statp.tile([1, 512], F32, tag="tmp", name="tmp")
                    nc.vector.tensor_tensor(tmp[:], sums[:], sums[:], ALU.mult)
                    nc.vector.tensor_scalar_mul(tmp[:], tmp[:], 1.0 / D)
                    nc.vector.tensor_tensor(tmp[:], psq[0:1, :], tmp[:], ALU.subtract)
                    sdt = statp.tile([1, 512], F32, tag="sdt", name="sdt")
                    nc.scalar.activation(
                        sdt[:], tmp[:], AF.Sqrt, bias=epst[0:1, :], scale=1.0 / D
                    )
                    rsd = statp.tile([1, 512], F32, tag="rsd", name="rsd")
                    nc.vector.reciprocal(rsd[:], sdt[:])
                    u = tmp
                    nc.vector.tensor_tensor(u[:], sums[:], rsd[:], ALU.mult)
                    nc.vector.tensor_scalar_mul(u[:], u[:], 1.0 / D)
                    RSb = lnbp.tile([P, 512], F32, tag="rsb", name="rsb")
                    nc.gpsimd.partition_broadcast(RSb[:], rsd[:])
                    Ub = lnbp.tile([P, 512], F32, tag="ub", name="ub")
                    nc.gpsimd.partition_broadcast(Ub[:], u[:])
                    for c in range(NCH):
                        nc.vector.tensor_tensor(
                            xn[:, c, sl], R[:, c, sl].bitcast(F32), RSb[:], ALU.mult
                        )
                        nc.vector.tensor_tensor(
                            xn[:, c, sl], xn[:, c, sl].bitcast(F32), Ub[:], ALU.subtract
                        )
                        if lw is not None:
                            g_sb, b_sb = lw
                            nc.vector.tensor_scalar(
                                xn[:, c, sl], xn[:, c, sl].bitcast(F32),
                                g_sb[:, c : c + 1], b_sb[:, c : c + 1],
                                ALU.mult, ALU.add,
                            )
                return xn

            def attention(xT, l, cross, lnw):
                """One attention sublayer; returns new xT."""
                if cross:
                    wq = load_w(wq_c, l, watp, F32R, "wat")
                    wk = load_w(wk_c, l, wcbp, BF16, "wcb")
                    wv = load_w(wv_c, l, wcbp, BF16, "wcb")
                    wo = load_w(wo_c, l, watp, F32R, "wat")
                    kvT, kv_dt = encT, BF16
                else:
                    wq = load_w(wq_s, l, watp, F32R, "wat")
                    wk = load_w(wk_s, l, watp, F32R, "wat")
                    wv = load_w(wv_s, l, watp, F32R, "wat")
                    wo = load_w(wo_s, l, watp, F32R, "wat")
                    kvT, kv_dt = xT, F32R

                QT = qkp.tile([P, NPAIR, T1], BF16, tag="qt")
                KT = qkp.tile([P, NPAIR, T1], BF16, tag="kt")
                # Q^T / K^T projections, head-pair col-tiled
                for pr in range(NPAIR):
                    for qc in range(NQC):
                        sl = slice(qc * 512, qc * 512 + 512)
                        for dst, w, src in ((QT, wq, xT), (KT, wk, kvT)):
                            ps = pp.tile([P, 512], F32, tag="pp")
                            for c in range(NCH):
                                nc.tensor.matmul(
                                    out=ps[:],
                                    lhsT=w[:, c, pr * P : (pr + 1) * P],
                                    rhs=src[:, c, sl],
                                    start=(c == 0), stop=(c == NCH - 1),
                                )
                            nc.vector.tensor_copy(dst[:, pr, sl], ps[:])
                # V (token-major, all heads) -> V_aug with ones column
                Vaug = vaugp.tile([P, H, NTT, 65], BF16, tag="vaug")
                nc.vector.memset(Vaug[:, :, :, 64:65], 1.0)
                for t in range(NTT):
                    psv = pp.tile([P, 512], F32, tag="pp")
                    for c in range(NCH):
                        nc.tensor.matmul(
                            out=psv[:],
                            lhsT=kvT[:, c, t * P : (t + 1) * P],
                            rhs=wv[:, c, :],
                            start=(c == 0), stop=(c == NCH - 1),
                        )
                    nc.vector.tensor_copy(
                        Vaug[:, :, t, 0:64],
                        psv[:].rearrange("p (h d) -> p h d", h=H),
                    )
                # attention core per pair
                Onorm = onormp.tile([P, NCH, T1], F32R, tag="onorm")
                for pr in range(NPAIR):
                    for qc in range(NQC):
                        sl = slice(qc * 512, qc * 512 + 512)
                        if causal and not cross:
                            kts = list(range(4 * (qc + 1)))
                        else:
                            kts = list(range(NTT))
                        psO = [
                            psOp.tile([65, 512], F32, tag="pso", name=f"psO0_{pr}_{qc}"),
                            psOp.tile([65, 512], F32, tag="pso", name=f"psO1_{pr}_{qc}"),
                        ]
                        for ti, t in enumerate(kts):
                            psS = psSp.tile([P, 2, 512], F32, tag="pss")
                            for h01 in range(2):
                                hsl = slice(h01 * 64, h01 * 64 + 64)
                                nc.tensor.matmul(
                                    out=psS[:, h01, :],
                                    lhsT=KT[hsl, pr, t * P : (t + 1) * P],
                                    rhs=QT[hsl, pr, sl],
                                    start=True, stop=True,
                                    tile_position=(64, 0) if h01 else None,
                                )
                            PT = ptp.tile([P, 2, 512], BF16, tag="pt")
                            nc.scalar.activation(
                                PT[:], psS[:], AF.Exp, bias=0.0, scale=0.125
                            )
                            if causal and not cross:
                                v = t - 4 * qc
                                if 0 <= v <= 3:
                                    nc.vector.tensor_tensor(
                                        PT[:], PT[:],
                                        tril4[:, v : v + 1, :].to_broadcast([P, 2, 512]),
                                        ALU.mult,
                                    )
                            elif not cross:
                                # generic self mask: transposed strided DMA + cast
                                mti = ptp.tile([P, 512], I32, tag="mti")
                                nc.sync.dma_start(
                                    mti[:],
                                    mask_in[sl, t * P : (t + 1) * P].rearrange(
                                        "q k -> k q"
                                    ),
                                )
                                mtf = ptp.tile([P, 1, 512], BF16, tag="mtf")
                                nc.vector.tensor_copy(mtf[:, 0, :], mti[:])
                                nc.vector.tensor_tensor(
                                    PT[:], PT[:],
                                    mtf[:, 0:1, :].to_broadcast([P, 2, 512]),
                                    ALU.mult,
                                )
                            if cross and not src_ones:
                                nc.vector.tensor_scalar_mul(
                                    PT[:], PT[:], smask_f[:, t : t + 1]
                                )
                            for h01 in range(2):
                                nc.tensor.matmul(
                                    out=psO[h01][:],
                                    lhsT=Vaug[:, 2 * pr + h01, t, :],
                                    rhs=PT[:, h01, :],
                                    start=(ti == 0), stop=(ti == len(kts) - 1),
                                )
                        for h01 in range(2):
                            rec = recp.tile([1, 512], F32, tag="rec")
                            nc.vector.reciprocal(rec[:], psO[h01][64:65, :])
                            recb = recp.tile([64, 512], F32, tag="recb")
                            nc.gpsimd.partition_broadcast(recb[:], rec[:])
                            nc.vector.tensor_tensor(
                                Onorm[h01 * 64 : h01 * 64 + 64, pr, sl],
                                psO[h01][0:64, :], recb[:], ALU.mult,
                            )
                # out-projection + residual
                R = xresp.tile([P, NCH, T1], F32R, tag="x")
                for dc in range(NCH):
                    for qc in range(NQC):
                        sl = slice(qc * 512, qc * 512 + 512)
                        psA = pp.tile([P, 512], F32, tag="pp")
                        for c in range(NCH):
                            nc.tensor.matmul(
                                out=psA[:],
                                lhsT=wo[:, c, dc * P : (dc + 1) * P],
                                rhs=Onorm[:, c, sl],
                                start=(c == 0), stop=(c == NCH - 1),
                            )
                        nc.vector.tensor_tensor(
                            R[:, dc, sl], psA[:], xT[:, dc, sl].bitcast(F32), ALU.add
                        )
                return feature_ln(R, lnw)

            def ffn(xT, l, lnw):
                w2t = w2p.tile([P, NDB, D], BF16, tag="w2")
                nc.sync.dma_start(
                    w2t[:], w2_in[l].rearrange("(c p) n -> p c n", p=P)
                )
                b1t = biasp.tile([P, NDB], F32, tag="b1")
                nc.sync.dma_start(b1t[:], b1_in[l])
                if not b2_zero:
                    b2t = biasp.tile([P, NCH], F32, tag="b2")
                    nc.sync.dma_start(b2t[:], b2_in[l])
                R = xresp.tile([P, NCH, T1], F32R, tag="x")
                for qc in range(NQC):
                    sl = slice(qc * 512, qc * 512 + 512)
                    Hq = []
                    w1q = None
                    for db in range(NDB):
                        if db % 4 == 0:
                            w1q = w1p.tile([P, NCH, 512], F32R, tag="w1", name=f"w1q{db}")
                            nc.sync.dma_start(
                                w1q[:],
                                w1_in[l, :, db * P : db * P + 512].rearrange(
                                    "(c p) n -> p c n", p=P
                                ),
                            )
                        psh = pp.tile([P, 512], F32, tag="pp")
                        off = (db % 4) * P
                        for c in range(NCH):
                            nc.tensor.matmul(
                                out=psh[:],
                                lhsT=w1q[:, c, off : off + P],
                                rhs=xT[:, c, sl],
                                start=(c == 0), stop=(c == NCH - 1),
                            )
                        ht = hp.tile([P, 512], BF16, tag="h")
                        if b1_zero:
                            nc.scalar.activation(ht[:], psh[:], AF.Relu)
                        else:
                            nc.scalar.activation(
                                ht[:], psh[:], AF.Relu, bias=b1t[:, db : db + 1]
                            )
                        Hq.append(ht)
                    for dc in range(NCH):
                        psf = pp.tile([P, 512], F32, tag="pp")
                        for db in range(NDB):
                            nc.tensor.matmul(
                                out=psf[:],
                                lhsT=w2t[:, db, dc * P : (dc + 1) * P],
                                rhs=Hq[db][:],
                                start=(db == 0), stop=(db == NDB - 1),
                            )
                        if b2_zero:
                            nc.vector.tensor_tensor(
                                R[:, dc, sl], psf[:], xT[:, dc, sl].bitcast(F32),
                                ALU.add,
                            )
                        else:
                            nc.vector.tensor_scalar(
                                R[:, dc, sl], psf[:], b2t[:, dc : dc + 1], None,
                                ALU.add,
                            )
                            nc.vector.tensor_tensor(
                                R[:, dc, sl], R[:, dc, sl].bitcast(F32),
                                xT[:, dc, sl].bitcast(F32), ALU.add,
                            )
                return feature_ln(R, lnw)

            # ================= layers =================
            def lnw_tiles(l, j):
                if ln_id:
                    return None
                g_sb = biasp.tile([P, NCH], F32, tag="lng")
                nc.sync.dma_start(g_sb[:], lng_in[l, j])
                b_sb = biasp.tile([P, NCH], F32, tag="lnbt")
                nc.sync.dma_start(b_sb[:], lnb_in[l, j])
                return (g_sb, b_sb)

            _ns = 3 * L if n_sub is None else n_sub
            subs = []
            for l in range(L):
                subs.append(("self", l))
                subs.append(("cross", l))
                subs.append(("ffn", l))
            for kind, l in subs[:_ns]:
                if kind == "self":
                    xT = attention(xT, l, cross=False, lnw=lnw_tiles(l, 0))
                elif kind == "cross":
                    xT = attention(xT, l, cross=True, lnw=lnw_tiles(l, 1))
                else:
                    xT = ffn(xT, l, lnw=lnw_tiles(l, 2))

            # ================= final transpose + store =================
            with tc.tile_pool(name="fin", bufs=3) as finp:
                for t in range(NTT):
                    ot = finp.tile([P, D], F32, tag="ot")
                    for c in range(NCH):
                        pst = pp.tile([P, 512], F32, tag="pp")
                        nc.tensor.transpose(
                            pst[:, 0:P], xT[:, c, t * P : (t + 1) * P].bitcast(F32),
                            ident[:],
                        )
                        nc.vector.tensor_copy(ot[:, c * P : (c + 1) * P], pst[:, 0:P])
                    nc.sync.dma_start(out_dram[:, t, :], ot[:])

    nc.compile()
    return nc


_PROGRAM_CACHE = {}


def kernel(**inputs):
    inp = {k: np.asarray(v) for k, v in inputs.items()}

    trg_seq = inp["trg_seq"].astype(np.int32, copy=False)
    trg_mask = inp["trg_mask"]
    enc_output = inp["enc_output"].astype(np.float32, copy=False)
    src_mask = inp["src_mask"]
    emb = inp["emb"].astype(np.float32, copy=False)

    tril = np.tril(np.ones((T1, T1), trg_mask.dtype))
    causal = bool(np.array_equal(trg_mask, np.broadcast_to(tril, trg_mask.shape)))
    src_ones = bool(np.all(src_mask == 1))
    ln_id = bool(
        np.all(inp["ln0_g"] == 1) and np.all(inp["ln0_b"] == 0)
        and all(np.all(inp[f"ln{j}_g"] == 1) for j in (1, 2, 3))
        and all(np.all(inp[f"ln{j}_b"] == 0) for j in (1, 2, 3))
    )
    b1_zero = bool(np.all(inp["b1"] == 0))
    b2_zero = bool(np.all(inp["b2"] == 0))

    key = (causal, src_ones, ln_id, b1_zero, b2_zero)
    if key not in _PROGRAM_CACHE:
        _PROGRAM_CACHE[key] = _build_program(*key)
    nc = _PROGRAM_CACHE[key]

    import ml_dtypes

    bf16 = ml_dtypes.bfloat16

    # ---- shared (batch-independent) device tensors ----
    pos_full = _sinusoid_table(NPOS, D)[:T1]  # [1024, 512] fp32
    pos_tiles = pos_full.reshape(NTT, P, D).transpose(1, 0, 2).copy()  # [P, NTT, D]

    # tril4[p, v, q] = 1 if p <= q - 128*v  (S^T diag-block mask variants)
    pgrid = np.arange(P)[:, None, None]
    vgrid = np.arange(4)[None, :, None]
    qgrid = np.arange(512)[None, None, :]
    tril4 = (pgrid <= qgrid - P * vgrid).astype(bf16)

    shared = {
        "emb": emb,
        "pos": pos_tiles,
        "tril4": tril4,
        "wq_s": inp["wq_s"].astype(np.float32, copy=False),
        "wk_s": inp["wk_s"].astype(np.float32, copy=False),
        "wv_s": inp["wv_s"].astype(np.float32, copy=False),
        "wo_s": inp["wo_s"].astype(np.float32, copy=False),
        "wq_c": inp["wq_c"].astype(np.float32, copy=False),
        "wk_c": inp["wk_c"].astype(bf16),
        "wv_c": inp["wv_c"].astype(bf16),
        "wo_c": inp["wo_c"].astype(np.float32, copy=False),
        "w1": inp["w1"].astype(np.float32, copy=False),
        "w2": inp["w2"].astype(bf16),
        "b1": inp["b1"].reshape(L, NDB, P).transpose(0, 2, 1).copy(),
        "b2": inp["b2"].reshape(L, NCH, P).transpose(0, 2, 1).copy(),
    }
    if not ln_id:
        shared["ln0_g"] = inp["ln0_g"].reshape(1, D).astype(np.float32, copy=False)
        shared["ln0_b"] = inp["ln0_b"].reshape(1, D).astype(np.float32, copy=False)
        shared["lng"] = np.stack(
            [inp[f"ln{j}_g"].reshape(L, NCH, P).transpose(0, 2, 1) for j in (1, 2, 3)],
            axis=1,
        ).copy()
        shared["lnb"] = np.stack(
            [inp[f"ln{j}_b"].reshape(L, NCH, P).transpose(0, 2, 1) for j in (1, 2, 3)],
            axis=1,
        ).copy()

    in_maps = []
    for b in range(B):
        m = dict(shared)
        idx = np.zeros((T1,), np.int32)
        idx[1:] = trg_seq[b]
        m["idx"] = idx.reshape(NTT, P).T.copy()  # [P, NTT]
        m["encT"] = np.ascontiguousarray(enc_output[b].T).reshape(NCH, P, S).transpose(1, 0, 2).copy().astype(bf16)
        if not causal:
            m["mask"] = np.ascontiguousarray(trg_mask[b]).astype(np.int32)
        if not src_ones:
            m["smask"] = src_mask[b, 0].astype(np.int32).reshape(NTT, P).T.copy()
        in_maps.append(m)

    res = run_bass_kernel_spmd(nc, in_maps, list(range(B)))

    out = np.empty((B, T1, D), np.float32)
    for b in range(B):
        o = res.results[b]["out"]  # [P, NTT, D]
        out[b] = o.transpose(1, 0, 2).reshape(T1, D)
    return out
